# revision 34
# baseline (speedup 1.0000x reference)
"""SAGAN-style attention block on 8 TRN2 NeuronCores, data-parallel over batch.

Per core (one batch b): x_b [C=256, N=4096] f32.
  q = Wq x + bq  [32, N];  k = Wk x + bk  [32, N]
  S = q^T k  [N, N];  attn = softmax(S, axis=0)  (column softmax over i)
  out = gamma * (v @ attn) + x,  v = Wv x + bv

Device algorithm (bf16 matmuls, f32 PSUM accumulation), v3:
  - no max-subtraction in softmax: |S| < ~50 empirically, exp() fits bf16
  - gamma folded into Wv/bv on the host: u = (gamma*v0)@E, so the
    normalize step is plain u * (1/colsum) with no gamma plumbing
  - x DMA'd in 8 slabs; slab-s projections start when slab s lands, the
    whole prefix is DMA-paced; 16 junk matmuls first so the PE p-state
    is warm (cold PE runs 2-4x slower and decays on any >~1us idle)
  - per wave: ONE 4-way row-tiled qk group into two fresh psum tiles of
    a 3-deep rotation, so qk only waits for the exp of wave w-2 (done
    ~1.5 waves earlier) and never gates the scalar engine; exp runs as
    two [P,2,JT] halves back-to-back (the wave pacer at ~2.2us)
  - v@E lags TWO waves: slots for wave w-2's kc-group run under wave
    w's exps
  - colsum off the tensor engine: one in-place [P,4,JT] DVE add per
    wave, GPSIMD partition_all_reduce (3.5us, replicated result, fully
    off-path) for the cross-partition sum, except the LAST tile which
    uses a ones-matmul on the then-idle tensor engine to shorten the
    tail
  - u PSUM is single-buffered per m-half; at each tile boundary the
    finished halves are drained to SBUF by the SCALAR engine (Copy)
    right after their last v@E slot -- the DVE is ~95% busy and would
    start these copies microseconds late, stalling the tensor engine
    (and dropping its p-state); normalization then runs from SBUF
    entirely off the critical path at mid-tile waves
"""

import numpy as np
import ml_dtypes

import concourse.bass as bass
import concourse.mybir as mybir
from concourse import bacc, tile
from concourse.bass import ds
from concourse.bass import bass_isa
from concourse.bass_utils import run_bass_kernel_spmd

F32 = mybir.dt.float32
BF16 = mybir.dt.bfloat16
AF = mybir.ActivationFunctionType
ALU = mybir.AluOpType

B, C, N = 8, 256, 4096
C8 = 32
P = 128
JT = 512          # j-tile width
NJT = N // JT     # 8 j-tiles
NKC = N // P      # 32 i/k chunks of 128
SLAB = 512        # input DMA/projection slab == j-tile width
NSLAB = N // SLAB

_cache = {}


def _build_nc():
    nc = bacc.Bacc("TRN2", target_bir_lowering=False, debug=False, num_devices=B)

    x16_d = nc.dram_tensor("x16", [P, NSLAB, 2, SLAB], BF16,
                           kind="ExternalInput").ap()
    wq_d = nc.dram_tensor("wq", [P, 2, P], BF16, kind="ExternalInput").ap()
    wk_d = nc.dram_tensor("wk", [P, 2, P], BF16, kind="ExternalInput").ap()
    wvt_d = nc.dram_tensor("wvt", [P, 2, C], BF16, kind="ExternalInput").ap()
    bq_d = nc.dram_tensor("bq", [P, 1], F32, kind="ExternalInput").ap()
    bk_d = nc.dram_tensor("bk", [P, 1], F32, kind="ExternalInput").ap()
    gbv_d = nc.dram_tensor("gbv", [P, 2], F32, kind="ExternalInput").ap()
    out_d = nc.dram_tensor("out", [C, N], F32, kind="ExternalOutput").ap()

    out_view = out_d.rearrange("(o p) n -> p o n", p=P)

    with tile.TileContext(nc) as tc:
        with (
            tc.tile_pool(name="const", bufs=1) as cpool,
            tc.tile_pool(name="big", bufs=1) as bigpool,
            tc.tile_pool(name="epool", bufs=2) as epool,
            tc.tile_pool(name="spool", bufs=2) as spool,
            tc.tile_pool(name="work", bufs=2) as wpool,
            tc.tile_pool(name="psQ", bufs=3, space="PSUM") as psQ,
            tc.tile_pool(name="psU", bufs=1, space="PSUM") as psU,
        ):
            # ---- DMAs: each dma_start costs ~1.5us of serial DGE
            # descriptor generation on the sequencer, so x goes in 4
            # grouped transfers sized to match the projection pipeline,
            # interleaved with the weights in dependency order
            ones_sb = cpool.tile([P, P], BF16, tag="ones")
            nc.vector.memset(ones_sb[:], 1.0)

            x16_s = [None] * NSLAB
            groups = [(0, 1), (1, 3), (3, 5), (5, 8)]
            gtiles = []

            def emit_x_group(gi):
                lo, hi = groups[gi]
                gt = bigpool.tile([P, hi - lo, 2, SLAB], BF16, tag=f"xg{gi}")
                nc.sync.dma_start(out=gt[:], in_=x16_d[:, lo:hi, :, :])
                for s in range(lo, hi):
                    x16_s[s] = gt[:, s - lo, :, :]
                gtiles.append(gt)

            emit_x_group(0)
            wq_sb = cpool.tile([P, 2, P], BF16, tag="wq")
            nc.sync.dma_start(out=wq_sb[:], in_=wq_d)
            wk_sb = cpool.tile([P, 2, P], BF16, tag="wk")
            nc.sync.dma_start(out=wk_sb[:], in_=wk_d)
            bq_sb = cpool.tile([P, 1], F32, tag="bq")
            nc.sync.dma_start(out=bq_sb[:], in_=bq_d)
            bk_sb = cpool.tile([P, 1], F32, tag="bk")
            nc.sync.dma_start(out=bk_sb[:], in_=bk_d)
            emit_x_group(1)
            wvt_sb = cpool.tile([P, 2, C], BF16, tag="wvt")
            nc.sync.dma_start(out=wvt_sb[:], in_=wvt_d)
            gbv_sb = cpool.tile([P, 2], F32, tag="gbv")
            nc.sync.dma_start(out=gbv_sb[:], in_=gbv_d)
            emit_x_group(2)
            emit_x_group(3)

            # preload the exp table during the DMA wait
            dummy = cpool.tile([P, 1], BF16, tag="dummy")
            nc.scalar.activation(dummy[:], bq_sb[:], AF.Exp)

            q_sb = bigpool.tile([P, N], BF16, tag="q")
            k_sb = bigpool.tile([P, N], BF16, tag="k")
            vt_sb = bigpool.tile([P, NKC, C], BF16, tag="vt")

            # PE p-state warmup: ~16 junk matmuls during the DMA wait so
            # the projections run at full clock (cold PE is 2-4x slower)
            warm = psQ.tile([P, 2, JT], F32, tag="qk", name="warm")
            for i in range(40):
                nc.tensor.matmul(
                    warm[:, 0, ds(0, P)],
                    ones_sb[:],
                    ones_sb[:],
                    start=True,
                    stop=True,
                )

            # ---- per-slab projections (emitted inside tile 0's waves) ----
            # q/k: out[m, n] = sum_c W_rep[c, m] * x16[c, n]  (M=128: 4
            # replicas of 32 so qk can row-tile 4-ways)
            # vt:  out[n, c] = sum_c' x16[c', n] * WvT[c', c]  (gamma folded
            # into WvT on the host)
            def emit_head_slab(s):
                xt = x16_s[s]
                sl = psQ.tile([P, 2, JT], F32, tag="qk", name=f"proj_{s}")
                for half, (w_sb, b_sb, dst) in enumerate(
                    ((wq_sb, bq_sb, q_sb), (wk_sb, bk_sb, k_sb))
                ):
                    for kc in range(2):
                        nc.tensor.matmul(
                            sl[:, half, :],
                            w_sb[:, kc, :],
                            xt[:, kc, :],
                            start=(kc == 0),
                            stop=(kc == 1),
                        )
                for half, (b_sb, dst) in enumerate(
                    ((bq_sb, q_sb), (bk_sb, k_sb))
                ):
                    nc.vector.tensor_scalar(
                        out=dst[:, ds(s * SLAB, SLAB)],
                        in0=sl[:, half, :],
                        scalar1=b_sb[:, :],
                        scalar2=None,
                        op0=ALU.add,
                    )
                # vt chunks 4s..4s+3, psum viewed as [P, 4, 256]
                vsl = psQ.tile([P, 2, JT], F32, tag="qk", name=f"vtp_{s}")
                vview = vsl[:].rearrange("p a (b c) -> p (a b) c", b=2, c=C)
                for c4 in range(4):
                    nck = 4 * s + c4
                    for kc in range(2):
                        nc.tensor.matmul(
                            vview[:, c4, :],
                            xt[:, kc, ds((nck % 4) * P, P)],
                            wvt_sb[:, kc, :],
                            start=(kc == 0),
                            stop=(kc == 1),
                        )
                # cast on the scalar engine: DVE is busy with the bias
                # adds, scalar is idle-ish in the head (Copy and Exp share
                # an activation table, so no table reload)
                nc.scalar.activation(
                    vt_sb[:, ds(4 * s, 4), :], vview, AF.Copy
                )

            # ---- main loop: 64 global waves + 4 tail waves ----
            # wave (t, w): qkA/qkB + exp0/exp1 for wave w of tile t;
            # v@E slots for global wave 8t+w-2; finalize pieces for tile
            # t-1 spread over waves 0..3.
            e_tiles = {}
            ep_tiles = {}
            esum_tiles = {}
            u_ps_t = {}
            r_tiles = {}

            def emit_ve_slots(g, half):
                # 4 col-tile pairs for global wave g's kc group, m = half
                t_v, w_v = divmod(g, 8)
                e_prev = e_tiles[t_v]
                u_ps = u_ps_t[t_v][half]
                m = half
                for kq in range(4):
                    kc = 4 * w_v + kq
                    for h in range(2):
                        nc.tensor.matmul(
                            u_ps[ds(64 * h, 64), :],
                            vt_sb[:, kc, ds(128 * m + 64 * h, 64)],
                            e_prev[:, kc, :],
                            start=(kc == 0),
                            stop=(kc == NKC - 1),
                            tile_position=(0, 64 * h),
                        )

            # projection prefix: slab s proj starts as soon as slab s's
            # DMA lands; the whole phase is DMA-paced (~16us) and runs at
            # full PE clock thanks to the warmup
            for s in range(NSLAB):
                emit_head_slab(s)

            uraw_tiles = {}

            def emit_finalize_piece(t, w):
                # tile t's colsum -> reciprocal -> normalize -> store,
                # spread over MID-tile waves of tile t+1 (3/5/6) so none
                # of this DVE work competes with the u-drain copies at the
                # tile boundary (waves 0-2), where any DVE delay stalls
                # the tensor engine and drops its p-state.
                if w == 0:
                    if t == NJT - 1:
                        # last tile: tensor engine is idle in the tail, so
                        # do colsum+broadcast as one ones-matmul chain
                        # instead of waiting on the 3.5us gpsimd all_reduce
                        sps = psQ.tile([P, 2, JT], F32, tag="qk",
                                       name=f"sps_{t}")
                        for i in range(4):
                            nc.tensor.matmul(
                                sps[:, 0, :],
                                ones_sb[:],
                                esum_tiles[t][:, i, :],
                                start=(i == 0),
                                stop=(i == 3),
                            )
                        r_tiles[t] = sps[:, 0, :]
                    else:
                        ep2 = spool.tile([P, 2, JT], BF16, tag="ep2",
                                         name=f"ep2_{t}")
                        nc.vector.tensor_add(
                            out=ep2[:],
                            in0=esum_tiles[t][:, ds(0, 2), :],
                            in1=esum_tiles[t][:, ds(2, 2), :],
                        )
                        esum = spool.tile([P, JT], BF16, tag="es1",
                                          name=f"es1_{t}")
                        nc.vector.tensor_add(
                            out=esum[:], in0=ep2[:, 0, :], in1=ep2[:, 1, :],
                        )
                        sall = wpool.tile([P, JT], F32, tag="sall",
                                          name=f"sall_{t}")
                        nc.gpsimd.partition_all_reduce(
                            sall[:], esum[:], 128, bass_isa.ReduceOp.add
                        )
                        r_tiles[t] = sall[:]
                elif w == 1:
                    r_sb = wpool.tile([P, JT], F32, tag="r", name=f"r_{t}")
                    nc.vector.reciprocal_approx_fast(out=r_sb[:], in_=r_tiles[t])
                    tmps = []
                    for m in range(2):
                        tmp = wpool.tile([P, JT], F32, tag=f"tmp{m}",
                                         name=f"tmp_{t}_{m}")
                        nc.vector.tensor_mul(
                            out=tmp[:], in0=uraw_tiles[t][m][:], in1=r_sb[:],
                        )
                        tmps.append(tmp)
                    r_tiles[t] = tmps
                elif w == 2:
                    outt = wpool.tile([P, 2, JT], F32, tag="outt", name=f"outt_{t}")
                    for m in range(2):
                        nc.vector.scalar_tensor_tensor(
                            out=outt[:, m, :],
                            in0=r_tiles[t][m][:],
                            scalar=gbv_sb[:, m, None],
                            in1=x16_s[t][:, m, :],
                            op0=ALU.add,
                            op1=ALU.add,
                        )
                    nc.sync.dma_start(
                        out=out_view[:, :, ds(t * JT, JT)], in_=outt[:]
                    )

            for t in range(NJT + 1):
                n_waves = 8 if t < NJT else 7
                if t < NJT:
                    e_tiles[t] = epool.tile(
                        [P, NKC, JT], BF16, tag="E", name=f"E_{t}"
                    )
                    esum_tiles[t] = spool.tile(
                        [P, 4, JT], BF16, tag="esum", name=f"esum_{t}"
                    )
                    js = ds(t * JT, JT)
                    e_cur = e_tiles[t]

                for w in range(n_waves):
                    g = 8 * t + w
                    # qk: one 4-way row-tiled group into two fresh psum
                    # tiles from the 3-deep rotation. The rotation means
                    # this group only waits for the exp of wave g-2, which
                    # finished ~1.5 waves ago -- qk never gates exp.
                    if t < NJT:
                        qa = psQ.tile([P, 2, JT], F32, tag="qk", name=f"qa_{g}")
                        qb = psQ.tile([P, 2, JT], F32, tag="qk", name=f"qb_{g}")
                        for r in range(4):
                            ic = 4 * w + r
                            nc.tensor.matmul(
                                (qa if r < 2 else qb)[:, r % 2, :],
                                q_sb[ds(32 * r, 32), ds(ic * P, P)],
                                k_sb[ds(32 * r, 32), js],
                                start=True,
                                stop=True,
                                tile_position=(32 * r, 0),
                            )
                    # finalize(t-1): fold+all_reduce at wave 3, recip+muls
                    # at wave 5 (all_reduce done by ~4.6), stt+dma at 6
                    if t >= 1 and t <= NJT and w in (3, 5, 6):
                        emit_finalize_piece(t - 1, {3: 0, 5: 1, 6: 2}[w])
                    if t < NJT and w == 2:
                        # u single-buffered per m-half: each allocation
                        # waits only on the matching drain copy of u(t-1),
                        # which ran under the OTHER half's v@E slots
                        u_ps_t[t] = [
                            psU.tile([P, JT], F32, tag=f"u{m}", name=f"u_{t}_{m}")
                            for m in range(2)
                        ]
                        uraw_tiles[t] = [None, None]
                    # v@E slots for wave g-2; afterwards drain the finished
                    # u(t-1) psum to SBUF on the (otherwise idle) GPSIMD
                    # engine -- the saturated DVE would start these copies
                    # microseconds late and stall the tensor engine
                    if 2 <= g < 66:
                        emit_ve_slots(g - 2, 0)
                        emit_ve_slots(g - 2, 1)
                        if t >= 1 and w == 1:
                            for m in range(2):
                                uraw = wpool.tile([P, JT], F32, tag=f"uraw{m}",
                                                  name=f"uraw_{t - 1}_{m}")
                                nc.scalar.activation(
                                    uraw[:], u_ps_t[t - 1][m][:], AF.Copy)
                                uraw_tiles[t - 1][m] = uraw
                    if t < NJT:
                        # exp in two halves: back-to-back on the scalar
                        # engine, each freeing its psum pair early
                        nc.scalar.activation(
                            e_cur[:, ds(4 * w, 2), :], qa[:], AF.Exp
                        )
                        nc.scalar.activation(
                            e_cur[:, ds(4 * w + 2, 2), :], qb[:], AF.Exp
                        )
                        # DVE colsum: one in-place [P, 4, JT] add per wave
                        if w == 0:
                            nc.vector.tensor_copy(
                                out=esum_tiles[t][:], in_=e_cur[:, ds(0, 4), :],
                            )
                        else:
                            nc.vector.tensor_add(
                                out=esum_tiles[t][:],
                                in0=esum_tiles[t][:],
                                in1=e_cur[:, ds(4 * w, 4), :],
                            )
    nc.compile()
    return nc


def _prep_inputs(x, Wq, bq, Wk, bk, Wv, bv, gamma):
    x = np.asarray(x, dtype=np.float32)
    Wq = np.asarray(Wq, dtype=np.float32)
    bq = np.asarray(bq, dtype=np.float32)
    Wk = np.asarray(Wk, dtype=np.float32)
    bk = np.asarray(bk, dtype=np.float32)
    Wv = np.asarray(Wv, dtype=np.float32)
    bv = np.asarray(bv, dtype=np.float32)
    g = float(np.asarray(gamma))

    bf = ml_dtypes.bfloat16
    # WqT replicated 4x along M so q lands replicated across 4x32 partitions
    wq_rep = np.tile(Wq.T, (1, 4)).reshape(2, P, P).transpose(1, 0, 2)
    wk_rep = np.tile(Wk.T, (1, 4)).reshape(2, P, P).transpose(1, 0, 2)
    # gamma folded into Wv / bv: u = (g*v) @ E, residual adds g*bv
    wvt = (g * Wv.T).reshape(2, P, C).transpose(1, 0, 2)
    bq_rep = np.tile(bq, 4)[:, None].astype(np.float32)
    bk_rep = np.tile(bk, 4)[:, None].astype(np.float32)
    gbv = (g * bv).reshape(2, P).T.copy().astype(np.float32)

    xf = x.reshape(B, C, N)
    x16 = xf.astype(bf)

    shared = {
        "wq": np.ascontiguousarray(wq_rep.astype(bf)),
        "wk": np.ascontiguousarray(wk_rep.astype(bf)),
        "wvt": np.ascontiguousarray(wvt.astype(bf)),
        "bq": bq_rep,
        "bk": bk_rep,
        "gbv": gbv,
    }
    in_maps = []
    for b in range(B):
        m = dict(shared)
        m["x16"] = np.ascontiguousarray(
            x16[b].reshape(2, P, NSLAB, SLAB).transpose(1, 2, 0, 3))
        in_maps.append(m)
    return in_maps


def _get_nc():
    if "nc" not in _cache:
        _cache["nc"] = _build_nc()
    return _cache["nc"]


def _install_neff_cache():
    """Cache compiled NEFFs by BIR hash: the bass_exec path skips the
    regular neuron compile cache, costing ~10min of walrus per process."""
    import hashlib
    import pathlib
    import shutil

    from concourse import bass2jax as b2j

    if getattr(b2j, "_ant_neff_cache_installed", False):
        return
    orig = b2j.compile_bir_kernel
    cache_dir = pathlib.Path("/root/.neuron-compile-cache/bass_neff")
    try:
        cache_dir.mkdir(parents=True, exist_ok=True)
    except OSError:
        return

    def cached(bir_json, tmpdir, neff_name="file.neff"):
        raw = bir_json if isinstance(bir_json, bytes) else bir_json.encode()
        h = hashlib.sha256(raw).hexdigest()
        hit = cache_dir / f"{h}.neff"
        if hit.exists():
            sg = pathlib.Path(tmpdir) / "sg00"
            sg.mkdir(parents=True, exist_ok=True)
            out = sg / neff_name
            shutil.copy(hit, out)
            return str(out)
        out = orig(bir_json, tmpdir, neff_name)
        try:
            shutil.copy(out, hit)
        except OSError:
            pass
        return out

    b2j.compile_bir_kernel = cached
    b2j._ant_neff_cache_installed = True


def _run(in_maps, trace=False):
    _install_neff_cache()
    nc = _get_nc()
    return run_bass_kernel_spmd(nc, in_maps, core_ids=list(range(B)), trace=trace)


def kernel(x, Wq, bq, Wk, bk, Wv, bv, gamma, _trace=False):
    x = np.asarray(x, dtype=np.float32)
    in_maps = _prep_inputs(x, Wq, bq, Wk, bk, Wv, bv, gamma)
    res = _run(in_maps, trace=_trace)
    out = np.stack([res.results[b]["out"] for b in range(B)])
    out = out.reshape(x.shape).astype(np.float32)
    if _trace:
        return out, res
    return out


def _enable_ntff_hook():
    """Register the axon NTFF profile hook (missing antenv.axon_hooks shim)."""
    import sys, types

    if "antenv.axon_hooks" in sys.modules:
        return
    mod = types.ModuleType("antenv.axon_hooks")
    mod._hook = None
    mod.set_axon_ntff_profile_hook = lambda h: setattr(mod, "_hook", h)
    mod.get_axon_ntff_profile_hook = lambda: mod._hook
    sys.modules["antenv.axon_hooks"] = mod
    import antenv

    antenv.axon_hooks = mod
    from trn_agent_boot.trn_boot import _ntff_profile_via_ctypes

    mod._hook = _ntff_profile_via_ctypes("/opt/axon/libaxon_pjrt.so")


# revision 35
# speedup vs baseline: 1.1686x; 1.1686x over previous
"""SAGAN-style attention block on 8 TRN2 NeuronCores, data-parallel over batch.

Per core (one batch b): x_b [C=256, N=4096] f32.
  q = Wq x + bq  [32, N];  k = Wk x + bk  [32, N]
  S = q^T k  [N, N];  attn = softmax(S, axis=0)  (column softmax over i)
  out = gamma * (v @ attn) + x,  v = Wv x + bv

Device algorithm (bf16 matmuls, f32 PSUM accumulation), v3:
  - no max-subtraction in softmax: |S| < ~50 empirically, exp() fits bf16
  - gamma folded into Wv/bv on the host: u = (gamma*v0)@E, so the
    normalize step is plain u * (1/colsum) with no gamma plumbing
  - x DMA'd in 8 slabs; slab-s projections start when slab s lands, the
    whole prefix is DMA-paced; 16 junk matmuls first so the PE p-state
    is warm (cold PE runs 2-4x slower and decays on any >~1us idle)
  - per wave: ONE 4-way row-tiled qk group into two fresh psum tiles of
    a 3-deep rotation, so qk only waits for the exp of wave w-2 (done
    ~1.5 waves earlier) and never gates the scalar engine; exp runs as
    two [P,2,JT] halves back-to-back (the wave pacer at ~2.2us)
  - v@E lags TWO waves: slots for wave w-2's kc-group run under wave
    w's exps
  - colsum off the tensor engine: one in-place [P,4,JT] DVE add per
    wave, GPSIMD partition_all_reduce (3.5us, replicated result, fully
    off-path) for the cross-partition sum, except the LAST tile which
    uses a ones-matmul on the then-idle tensor engine to shorten the
    tail
  - u PSUM is single-buffered per m-half; at each tile boundary the
    finished halves are drained to SBUF by the SCALAR engine (Copy)
    right after their last v@E slot -- the DVE is ~95% busy and would
    start these copies microseconds late, stalling the tensor engine
    (and dropping its p-state); normalization then runs from SBUF
    entirely off the critical path at mid-tile waves
"""

import numpy as np
import ml_dtypes

import concourse.bass as bass
import concourse.mybir as mybir
from concourse import bacc, tile
from concourse.bass import ds
from concourse.bass import bass_isa
from concourse.bass_utils import run_bass_kernel_spmd

F32 = mybir.dt.float32
BF16 = mybir.dt.bfloat16
AF = mybir.ActivationFunctionType
ALU = mybir.AluOpType

B, C, N = 8, 256, 4096
C8 = 32
P = 128
JT = 512          # j-tile width
NJT = N // JT     # 8 j-tiles
NKC = N // P      # 32 i/k chunks of 128
SLAB = 512        # input DMA/projection slab == j-tile width
NSLAB = N // SLAB

_cache = {}


def _build_nc():
    nc = bacc.Bacc("TRN2", target_bir_lowering=False, debug=False, num_devices=B)

    x16_d = nc.dram_tensor("x16", [P, NSLAB, 2, SLAB], BF16,
                           kind="ExternalInput").ap()
    wq_d = nc.dram_tensor("wq", [P, 2, P], BF16, kind="ExternalInput").ap()
    wk_d = nc.dram_tensor("wk", [P, 2, P], BF16, kind="ExternalInput").ap()
    wvt_d = nc.dram_tensor("wvt", [P, 2, C], BF16, kind="ExternalInput").ap()
    bq_d = nc.dram_tensor("bq", [P, 1], F32, kind="ExternalInput").ap()
    bk_d = nc.dram_tensor("bk", [P, 1], F32, kind="ExternalInput").ap()
    gbv_d = nc.dram_tensor("gbv", [P, 2], F32, kind="ExternalInput").ap()
    out_d = nc.dram_tensor("out", [C, N], F32, kind="ExternalOutput").ap()

    out_view = out_d.rearrange("(o p) n -> p o n", p=P)

    with tile.TileContext(nc) as tc:
        with (
            tc.tile_pool(name="const", bufs=1) as cpool,
            tc.tile_pool(name="big", bufs=1) as bigpool,
            tc.tile_pool(name="epool", bufs=2) as epool,
            tc.tile_pool(name="spool", bufs=2) as spool,
            tc.tile_pool(name="work", bufs=2) as wpool,
            tc.tile_pool(name="psQ", bufs=3, space="PSUM") as psQ,
            tc.tile_pool(name="psU", bufs=1, space="PSUM") as psU,
        ):
            # ---- DMAs: each dma_start costs ~1.5us of serial DGE
            # descriptor generation on the sequencer, so x goes in 4
            # grouped transfers sized to match the projection pipeline,
            # interleaved with the weights in dependency order
            x16_s = [None] * NSLAB
            groups = [(0, 1), (1, 3), (3, 5), (5, 8)]
            gtiles = []

            def emit_x_group(gi):
                lo, hi = groups[gi]
                gt = bigpool.tile([P, hi - lo, 2, SLAB], BF16, tag=f"xg{gi}")
                nc.sync.dma_start(out=gt[:], in_=x16_d[:, lo:hi, :, :])
                for s in range(lo, hi):
                    x16_s[s] = gt[:, s - lo, :, :]
                gtiles.append(gt)

            emit_x_group(0)
            wq_sb = cpool.tile([P, 2, P], BF16, tag="wq")
            nc.sync.dma_start(out=wq_sb[:], in_=wq_d)
            wk_sb = cpool.tile([P, 2, P], BF16, tag="wk")
            nc.sync.dma_start(out=wk_sb[:], in_=wk_d)
            bq_sb = cpool.tile([P, 1], F32, tag="bq")
            nc.sync.dma_start(out=bq_sb[:], in_=bq_d)
            bk_sb = cpool.tile([P, 1], F32, tag="bk")
            nc.sync.dma_start(out=bk_sb[:], in_=bk_d)
            emit_x_group(1)
            wvt_sb = cpool.tile([P, 2, C], BF16, tag="wvt")
            nc.sync.dma_start(out=wvt_sb[:], in_=wvt_d)
            gbv_sb = cpool.tile([P, 2], F32, tag="gbv")
            nc.sync.dma_start(out=gbv_sb[:], in_=gbv_d)
            emit_x_group(2)
            emit_x_group(3)
            ones_sb = cpool.tile([P, P], BF16, tag="ones")
            nc.vector.memset(ones_sb[:], 1.0)

            # preload the exp table during the DMA wait
            dummy = cpool.tile([P, 1], BF16, tag="dummy")
            nc.scalar.activation(dummy[:], bq_sb[:], AF.Exp)

            q_sb = bigpool.tile([P, N], BF16, tag="q")
            k_sb = bigpool.tile([P, N], BF16, tag="k")
            vt_sb = bigpool.tile([P, NKC, C], BF16, tag="vt")

            # PE p-state warmup: ~16 junk matmuls during the DMA wait so
            # the projections run at full clock (cold PE is 2-4x slower)
            warm = psQ.tile([P, 2, JT], F32, tag="qk", name="warm")
            for i in range(16):
                nc.tensor.matmul(
                    warm[:, 0, ds(0, C)],
                    wq_sb[:, 0, :],
                    wq_sb[:].rearrange("p a b -> p (a b)"),
                    start=True,
                    stop=True,
                )

            # ---- per-slab projections (emitted inside tile 0's waves) ----
            # q/k: out[m, n] = sum_c W_rep[c, m] * x16[c, n]  (M=128: 4
            # replicas of 32 so qk can row-tile 4-ways)
            # vt:  out[n, c] = sum_c' x16[c', n] * WvT[c', c]  (gamma folded
            # into WvT on the host)
            def emit_head_slab(s):
                xt = x16_s[s]
                sl = psQ.tile([P, 2, JT], F32, tag="qk", name=f"proj_{s}")
                for half, (w_sb, b_sb, dst) in enumerate(
                    ((wq_sb, bq_sb, q_sb), (wk_sb, bk_sb, k_sb))
                ):
                    for kc in range(2):
                        nc.tensor.matmul(
                            sl[:, half, :],
                            w_sb[:, kc, :],
                            xt[:, kc, :],
                            start=(kc == 0),
                            stop=(kc == 1),
                        )
                for half, (b_sb, dst) in enumerate(
                    ((bq_sb, q_sb), (bk_sb, k_sb))
                ):
                    nc.vector.tensor_scalar(
                        out=dst[:, ds(s * SLAB, SLAB)],
                        in0=sl[:, half, :],
                        scalar1=b_sb[:, :],
                        scalar2=None,
                        op0=ALU.add,
                    )
                # vt chunks 4s..4s+3, psum viewed as [P, 4, 256]
                vsl = psQ.tile([P, 2, JT], F32, tag="qk", name=f"vtp_{s}")
                vview = vsl[:].rearrange("p a (b c) -> p (a b) c", b=2, c=C)
                for c4 in range(4):
                    nck = 4 * s + c4
                    for kc in range(2):
                        nc.tensor.matmul(
                            vview[:, c4, :],
                            xt[:, kc, ds((nck % 4) * P, P)],
                            wvt_sb[:, kc, :],
                            start=(kc == 0),
                            stop=(kc == 1),
                        )
                # cast on the scalar engine: DVE is busy with the bias
                # adds, scalar is idle-ish in the head (Copy and Exp share
                # an activation table, so no table reload)
                nc.scalar.activation(
                    vt_sb[:, ds(4 * s, 4), :], vview, AF.Copy
                )

            # ---- main loop: 64 global waves + 4 tail waves ----
            # wave (t, w): qkA/qkB + exp0/exp1 for wave w of tile t;
            # v@E slots for global wave 8t+w-2; finalize pieces for tile
            # t-1 spread over waves 0..3.
            e_tiles = {}
            ep_tiles = {}
            esum_tiles = {}
            u_ps_t = {}
            r_tiles = {}

            def emit_ve_slots(g, half):
                # 4 col-tile pairs for global wave g's kc group, m = half
                t_v, w_v = divmod(g, 8)
                e_prev = e_tiles[t_v]
                u_ps = u_ps_t[t_v][half]
                m = half
                for kq in range(4):
                    kc = 4 * w_v + kq
                    for h in range(2):
                        nc.tensor.matmul(
                            u_ps[ds(64 * h, 64), :],
                            vt_sb[:, kc, ds(128 * m + 64 * h, 64)],
                            e_prev[:, kc, :],
                            start=(kc == 0),
                            stop=(kc == NKC - 1),
                            tile_position=(0, 64 * h),
                        )

            # projection prefix: slab s proj starts as soon as slab s's
            # DMA lands; the whole phase is DMA-paced (~16us) and runs at
            # full PE clock thanks to the warmup
            for s in range(NSLAB):
                emit_head_slab(s)

            uraw_tiles = {}

            def emit_finalize_piece(t, w):
                # tile t's colsum -> reciprocal -> normalize -> store,
                # spread over MID-tile waves of tile t+1 (3/5/6) so none
                # of this DVE work competes with the u-drain copies at the
                # tile boundary (waves 0-2), where any DVE delay stalls
                # the tensor engine and drops its p-state.
                if w == 0:
                    if t == NJT - 1:
                        # last tile: tensor engine is idle in the tail, so
                        # do colsum+broadcast as one ones-matmul chain
                        # instead of waiting on the 3.5us gpsimd all_reduce
                        sps = psQ.tile([P, 2, JT], F32, tag="qk",
                                       name=f"sps_{t}")
                        for i in range(4):
                            nc.tensor.matmul(
                                sps[:, 0, :],
                                ones_sb[:],
                                esum_tiles[t][:, i, :],
                                start=(i == 0),
                                stop=(i == 3),
                            )
                        r_tiles[t] = sps[:, 0, :]
                    else:
                        ep2 = spool.tile([P, 2, JT], BF16, tag="ep2",
                                         name=f"ep2_{t}")
                        nc.vector.tensor_add(
                            out=ep2[:],
                            in0=esum_tiles[t][:, ds(0, 2), :],
                            in1=esum_tiles[t][:, ds(2, 2), :],
                        )
                        esum = spool.tile([P, JT], BF16, tag="es1",
                                          name=f"es1_{t}")
                        nc.vector.tensor_add(
                            out=esum[:], in0=ep2[:, 0, :], in1=ep2[:, 1, :],
                        )
                        sall = wpool.tile([P, JT], F32, tag="sall",
                                          name=f"sall_{t}")
                        nc.gpsimd.partition_all_reduce(
                            sall[:], esum[:], 128, bass_isa.ReduceOp.add
                        )
                        r_tiles[t] = sall[:]
                elif w == 1:
                    r_sb = wpool.tile([P, JT], F32, tag="r", name=f"r_{t}")
                    nc.vector.reciprocal_approx_fast(out=r_sb[:], in_=r_tiles[t])
                    tmps = []
                    for m in range(2):
                        tmp = wpool.tile([P, JT], F32, tag=f"tmp{m}",
                                         name=f"tmp_{t}_{m}")
                        nc.vector.tensor_mul(
                            out=tmp[:], in0=uraw_tiles[t][m][:], in1=r_sb[:],
                        )
                        tmps.append(tmp)
                    r_tiles[t] = tmps
                elif w == 2:
                    outt = wpool.tile([P, 2, JT], F32, tag="outt", name=f"outt_{t}")
                    for m in range(2):
                        nc.vector.scalar_tensor_tensor(
                            out=outt[:, m, :],
                            in0=r_tiles[t][m][:],
                            scalar=gbv_sb[:, m, None],
                            in1=x16_s[t][:, m, :],
                            op0=ALU.add,
                            op1=ALU.add,
                        )
                    nc.sync.dma_start(
                        out=out_view[:, :, ds(t * JT, JT)], in_=outt[:]
                    )

            for t in range(NJT + 1):
                n_waves = 8 if t < NJT else 7
                if t < NJT:
                    e_tiles[t] = epool.tile(
                        [P, NKC, JT], BF16, tag="E", name=f"E_{t}"
                    )
                    esum_tiles[t] = spool.tile(
                        [P, 4, JT], BF16, tag="esum", name=f"esum_{t}"
                    )
                    js = ds(t * JT, JT)
                    e_cur = e_tiles[t]

                for w in range(n_waves):
                    g = 8 * t + w
                    # qk: one 4-way row-tiled group into two fresh psum
                    # tiles from the 3-deep rotation. The rotation means
                    # this group only waits for the exp of wave g-2, which
                    # finished ~1.5 waves ago -- qk never gates exp.
                    if t < NJT:
                        qa = psQ.tile([P, 2, JT], F32, tag="qk", name=f"qa_{g}")
                        qb = psQ.tile([P, 2, JT], F32, tag="qk", name=f"qb_{g}")
                        for r in range(4):
                            ic = 4 * w + r
                            nc.tensor.matmul(
                                (qa if r < 2 else qb)[:, r % 2, :],
                                q_sb[ds(32 * r, 32), ds(ic * P, P)],
                                k_sb[ds(32 * r, 32), js],
                                start=True,
                                stop=True,
                                tile_position=(32 * r, 0),
                            )
                    # finalize(t-1): fold+all_reduce at wave 3, recip+muls
                    # at wave 5 (all_reduce done by ~4.6), stt+dma at 6
                    if t >= 1 and t <= NJT and w in (3, 5, 6):
                        emit_finalize_piece(t - 1, {3: 0, 5: 1, 6: 2}[w])
                    if t < NJT and w == 2:
                        # u single-buffered per m-half: each allocation
                        # waits only on the matching drain copy of u(t-1),
                        # which ran under the OTHER half's v@E slots
                        u_ps_t[t] = [
                            psU.tile([P, JT], F32, tag=f"u{m}", name=f"u_{t}_{m}")
                            for m in range(2)
                        ]
                        uraw_tiles[t] = [None, None]
                    # v@E slots for wave g-2; afterwards drain the finished
                    # u(t-1) psum to SBUF on the (otherwise idle) GPSIMD
                    # engine -- the saturated DVE would start these copies
                    # microseconds late and stall the tensor engine
                    if 2 <= g < 66:
                        emit_ve_slots(g - 2, 0)
                        emit_ve_slots(g - 2, 1)
                        if t >= 1 and w == 1:
                            for m in range(2):
                                uraw = wpool.tile([P, JT], F32, tag=f"uraw{m}",
                                                  name=f"uraw_{t - 1}_{m}")
                                nc.scalar.activation(
                                    uraw[:], u_ps_t[t - 1][m][:], AF.Copy)
                                uraw_tiles[t - 1][m] = uraw
                    if t < NJT:
                        # exp in two halves: back-to-back on the scalar
                        # engine, each freeing its psum pair early
                        nc.scalar.activation(
                            e_cur[:, ds(4 * w, 2), :], qa[:], AF.Exp
                        )
                        nc.scalar.activation(
                            e_cur[:, ds(4 * w + 2, 2), :], qb[:], AF.Exp
                        )
                        # DVE colsum: one in-place [P, 4, JT] add per wave
                        if w == 0:
                            nc.vector.tensor_copy(
                                out=esum_tiles[t][:], in_=e_cur[:, ds(0, 4), :],
                            )
                        else:
                            nc.vector.tensor_add(
                                out=esum_tiles[t][:],
                                in0=esum_tiles[t][:],
                                in1=e_cur[:, ds(4 * w, 4), :],
                            )
    nc.compile()
    return nc


def _prep_inputs(x, Wq, bq, Wk, bk, Wv, bv, gamma):
    x = np.asarray(x, dtype=np.float32)
    Wq = np.asarray(Wq, dtype=np.float32)
    bq = np.asarray(bq, dtype=np.float32)
    Wk = np.asarray(Wk, dtype=np.float32)
    bk = np.asarray(bk, dtype=np.float32)
    Wv = np.asarray(Wv, dtype=np.float32)
    bv = np.asarray(bv, dtype=np.float32)
    g = float(np.asarray(gamma))

    bf = ml_dtypes.bfloat16
    # WqT replicated 4x along M so q lands replicated across 4x32 partitions
    wq_rep = np.tile(Wq.T, (1, 4)).reshape(2, P, P).transpose(1, 0, 2)
    wk_rep = np.tile(Wk.T, (1, 4)).reshape(2, P, P).transpose(1, 0, 2)
    # gamma folded into Wv / bv: u = (g*v) @ E, residual adds g*bv
    wvt = (g * Wv.T).reshape(2, P, C).transpose(1, 0, 2)
    bq_rep = np.tile(bq, 4)[:, None].astype(np.float32)
    bk_rep = np.tile(bk, 4)[:, None].astype(np.float32)
    gbv = (g * bv).reshape(2, P).T.copy().astype(np.float32)

    xf = x.reshape(B, C, N)
    x16 = xf.astype(bf)

    shared = {
        "wq": np.ascontiguousarray(wq_rep.astype(bf)),
        "wk": np.ascontiguousarray(wk_rep.astype(bf)),
        "wvt": np.ascontiguousarray(wvt.astype(bf)),
        "bq": bq_rep,
        "bk": bk_rep,
        "gbv": gbv,
    }
    in_maps = []
    for b in range(B):
        m = dict(shared)
        m["x16"] = np.ascontiguousarray(
            x16[b].reshape(2, P, NSLAB, SLAB).transpose(1, 2, 0, 3))
        in_maps.append(m)
    return in_maps


def _get_nc():
    if "nc" not in _cache:
        _cache["nc"] = _build_nc()
    return _cache["nc"]


def _install_neff_cache():
    """Cache compiled NEFFs by BIR hash: the bass_exec path skips the
    regular neuron compile cache, costing ~10min of walrus per process."""
    import hashlib
    import pathlib
    import shutil

    from concourse import bass2jax as b2j

    if getattr(b2j, "_ant_neff_cache_installed", False):
        return
    orig = b2j.compile_bir_kernel
    cache_dir = pathlib.Path("/root/.neuron-compile-cache/bass_neff")
    try:
        cache_dir.mkdir(parents=True, exist_ok=True)
    except OSError:
        return

    def cached(bir_json, tmpdir, neff_name="file.neff"):
        raw = bir_json if isinstance(bir_json, bytes) else bir_json.encode()
        h = hashlib.sha256(raw).hexdigest()
        hit = cache_dir / f"{h}.neff"
        if hit.exists():
            sg = pathlib.Path(tmpdir) / "sg00"
            sg.mkdir(parents=True, exist_ok=True)
            out = sg / neff_name
            shutil.copy(hit, out)
            return str(out)
        out = orig(bir_json, tmpdir, neff_name)
        try:
            shutil.copy(out, hit)
        except OSError:
            pass
        return out

    b2j.compile_bir_kernel = cached
    b2j._ant_neff_cache_installed = True


def _run(in_maps, trace=False):
    _install_neff_cache()
    nc = _get_nc()
    return run_bass_kernel_spmd(nc, in_maps, core_ids=list(range(B)), trace=trace)


def kernel(x, Wq, bq, Wk, bk, Wv, bv, gamma, _trace=False):
    x = np.asarray(x, dtype=np.float32)
    in_maps = _prep_inputs(x, Wq, bq, Wk, bk, Wv, bv, gamma)
    res = _run(in_maps, trace=_trace)
    out = np.stack([res.results[b]["out"] for b in range(B)])
    out = out.reshape(x.shape).astype(np.float32)
    if _trace:
        return out, res
    return out


def _enable_ntff_hook():
    """Register the axon NTFF profile hook (missing antenv.axon_hooks shim)."""
    import sys, types

    if "antenv.axon_hooks" in sys.modules:
        return
    mod = types.ModuleType("antenv.axon_hooks")
    mod._hook = None
    mod.set_axon_ntff_profile_hook = lambda h: setattr(mod, "_hook", h)
    mod.get_axon_ntff_profile_hook = lambda: mod._hook
    sys.modules["antenv.axon_hooks"] = mod
    import antenv

    antenv.axon_hooks = mod
    from trn_agent_boot.trn_boot import _ntff_profile_via_ctypes

    mod._hook = _ntff_profile_via_ctypes("/opt/axon/libaxon_pjrt.so")
